# revision 11
# baseline (speedup 1.0000x reference)
"""Trainium2 Bass kernel for nn_CapsuleLayer (dynamic routing capsule layer).

Sharding: the 1152 input capsules (i) are split across 8 cores (144 each);
the full batch B=128 lives on SBUF partitions. Routing state (c, p) stays
local to each core's i-shard; per-iteration s partial sums are combined with
small AllReduces ([128,80] f32, one per j-half so the collective latency
hides under the other half's compute). u_hat is never materialized:
  s[b,j,d]       = sum_{i,k} (exp(c)/sigma)[b,j,i] x[b,i,k] W[j,i,d,k]  (PE)
  c_delta[b,j,i] = sum_k x[b,i,k] m[b,j,i,k],   m = sum_d v[b,j,d] W[j,i,d,k]
Host pre-lays W out in three device layouts (i-major bf16 for s, (i*k)-packed
bf16 for the 16-capsule remainder chunk, d-major f32 for m) so the device
does no staging casts. All activation funcs (exp, ln for rsqrt) come from
one table set, so LoadActFuncSet is emitted once.
"""

import sys

if "/opt/trn_rl_repo" not in sys.path:
    sys.path.insert(0, "/opt/trn_rl_repo")

import contextlib

import numpy as np

import concourse.bass as bass  # noqa: F401
import concourse.tile as tile
from concourse import bacc, mybir
from concourse.bass_utils import run_bass_kernel_spmd
from concourse.masks import make_identity

f32 = mybir.dt.float32
f32r = mybir.dt.float32r
bf16 = mybir.dt.bfloat16
AL = mybir.AluOpType
AF = mybir.ActivationFunctionType

B = 128          # batch (on partitions)
NJ = 10          # output capsules
DO = 16          # output capsule dim
DI = 8           # input capsule dim
NI = 1152        # input capsules (global)
ROUTINGS = 3
EPS = 1e-7

N0 = 128         # chunk0: i = 0..127, i on partitions
N1 = 16          # chunk1: i = 128..143, (i,k) packed on partitions (16*8=128)
M_SL = 3         # m-matmul free-dim slices (3 x 384 = 1152)
M_FREE = (N0 + N1) * DI // M_SL

# engine assignment knobs (tuned against TimelineSim)
# t-multiply mode per j: A = DVE direct from PSUM, B = Act evac + DVE mult,
# D = Pool direct from PSUM
T_MODE = "BBDAB" * 2
Y_POOL_J = (3, 6, 9)         # j's whose chunk0 y-multiply runs on Pool


def _stt(eng, out, in0, in1, op1, scalar=None, op0=None):
    if scalar is not None and scalar != 1.0 or op0 is not None and op0 != AL.mult:
        eng.scalar_tensor_tensor(out=out, in0=in0, scalar=scalar or 1.0,
                                 in1=in1, op0=op0 or AL.mult, op1=op1)
    else:
        eng.tensor_tensor(out=out, in0=in0, in1=in1, op=op1)


def build_kernel(n_cores=8, debug=False, repeat=1, single=False, ablate=()):
    ni_l = NI // n_cores
    assert ni_l == N0 + N1

    nc = bacc.Bacc("TRN2", target_bir_lowering=False, debug=False,
                   num_devices=1 if single else n_cores)
    # host-prepped layouts (see kernel() below)
    x_d = nc.dram_tensor("x", [B, ni_l, DI], bf16, kind="ExternalInput")
    w0_d = nc.dram_tensor("w0", [N0, NJ, DO, DI], bf16, kind="ExternalInput")
    w1_d = nc.dram_tensor("w1", [N1 * DI, NJ, DO], bf16, kind="ExternalInput")
    wd_d = nc.dram_tensor("wd", [DO, NJ, ni_l, DI], f32, kind="ExternalInput")
    out_d = nc.dram_tensor("out", [B, NJ, DO], f32, kind="ExternalOutput")

    with tile.TileContext(nc) as tc:
        for _rep in range(repeat):
            _body(nc, tc, x_d, w0_d, w1_d, wd_d, out_d, ni_l, n_cores, single)
    nc.compile()
    return nc


def _body(nc, tc, x_d, w0_d, w1_d, wd_d, out_d, ni_l, n_cores, single=False):
    ctx = contextlib.ExitStack()
    JH = NJ // 2
    with ctx:
        sb = ctx.enter_context(tc.tile_pool(name="sb", bufs=1))
        sc = ctx.enter_context(tc.tile_pool(name="scratch", bufs=2))
        ps = ctx.enter_context(tc.tile_pool(name="ps", bufs=2, space="PSUM"))
        dram = ctx.enter_context(tc.tile_pool(name="dram", bufs=2, space="DRAM"))

        # ---------------- loads (no staging: host sent final layouts) -------
        x_bf = sb.tile([B, ni_l, DI], bf16)
        nc.sync.dma_start(out=x_bf.rearrange("b i k -> b (i k)"),
                          in_=x_d.ap().rearrange("b i k -> b (i k)"))
        w0 = sb.tile([N0, NJ, DO, DI], bf16)
        nc.sync.dma_start(out=w0.rearrange("i j d k -> i (j d k)"),
                          in_=w0_d.ap().rearrange("i j d k -> i (j d k)"))
        w1 = sb.tile([N1 * DI, NJ, DO], bf16)
        nc.sync.dma_start(out=w1.rearrange("p j d -> p (j d)"),
                          in_=w1_d.ap().rearrange("p j d -> p (j d)"))
        wd = sb.tile([DO, NJ, ni_l, DI], f32)
        nc.sync.dma_start(out=wd.rearrange("d j i k -> d (j i k)"),
                          in_=wd_d.ap().rearrange("d j i k -> d (j i k)"))
        wd_r = wd.bitcast(f32r)

        ident = sb.tile([128, 128], bf16)
        make_identity(nc, ident)
        ident_f = sb.tile([128, 128], f32)
        make_identity(nc, ident_f)

        # x_P0: [i0, k, b]; x_P1: [(i1 k), b]  (PE transposes via PSUM)
        x_P0 = sb.tile([N0, DI, B], bf16)
        x_P1 = sb.tile([N1 * DI, B], bf16)
        for k in range(DI):
            pt = ps.tile([128, B], bf16, tag="tp", name="pt", padded_shape=[128, 1024])
            nc.tensor.transpose(pt, x_bf[:, 0:N0, k], ident)
            eng = nc.vector if k % 2 == 0 else nc.scalar
            if eng is nc.scalar:
                eng.copy(out=x_P0[:, k, :], in_=pt)
            else:
                eng.tensor_copy(out=x_P0[:, k, :], in_=pt)
        pt = ps.tile([128, B], bf16, tag="tp", name="pt", padded_shape=[128, 1024])
        nc.tensor.transpose(
            pt, x_bf[:, N0:ni_l, :].rearrange("b i k -> b (i k)"), ident)
        nc.vector.tensor_copy(out=x_P1, in_=pt)

        # persistent state
        c_t = sb.tile([B, NJ, ni_l], f32)        # routing logits
        s_full = sb.tile([B, NJ, DO], f32)       # all-reduced s (j,d)
        v_jd = sb.tile([B, NJ, DO], f32)         # squashed v (j,d)
        v_T = sb.tile([DO, NJ, B], f32r)         # v transposed [d, j, b]
        e_bf = sb.tile([B, NJ, ni_l], bf16)      # exp(c)
        sg1 = sb.tile([B, 5, ni_l], f32)         # sigma tree scratch
        rin = sb.tile([B, ni_l], f32)            # 1/sigma
        rin_bf = sb.tile([B, ni_l], bf16)
        rT0 = sb.tile([N0, B], bf16)
        rT1 = sb.tile([N1 * DI, B], bf16)
        xs0 = sb.tile([N0, DI, B], bf16)         # x * (1/sigma), chunk0
        xs1 = sb.tile([N1 * DI, B], bf16)
        y0 = sb.tile([N0, DI, B], bf16)
        y1 = sb.tile([N1 * DI, B], bf16)
        t_all = sb.tile([B, NJ, ni_l, DI], bf16)  # m * x scratch
        sq = sb.tile([B, NJ], f32)
        fac = sb.tile([B, NJ], f32)
        v_out = sb.tile([B, NJ, DO], f32)
        eps_t = sb.tile([B, 1], f32)
        nc.vector.memset(eps_t, EPS)

        ar_in = [dram.tile([B, JH * DO], f32, tag=f"ai{h}", name=f"ai{h}") for h in range(2)]
        ar_out = [dram.tile([B, JH * DO], f32, tag=f"ao{h}", name=f"ao{h}") for h in range(2)]

        s_part = [sb.tile([B, JH * DO], f32, tag=f"sp{h}", name=f"sp{h}")
                  for h in range(2)]

        def allreduce_h(jh, ps_h):
            # PSUM -> SBUF -> DRAM -> (collective) -> SBUF half of s_full
            nc.scalar.copy(out=s_part[jh],
                           in_=ps_h.rearrange("b j d -> b (j d)"))
            nc.sync.dma_start(out=ar_in[jh], in_=s_part[jh])
            if single:
                nc.sync.dma_start(out=ar_out[jh], in_=ar_in[jh])
            else:
                nc.gpsimd.collective_compute(
                    "AllReduce", AL.add,
                    ins=[ar_in[jh].opt()], outs=[ar_out[jh].opt()],
                    replica_groups=[list(range(n_cores))],
                )
            jsl = slice(jh * JH, (jh + 1) * JH)
            nc.sync.dma_start(
                out=s_full[:, jsl, :].rearrange("b j d -> b (j d)"),
                in_=ar_out[jh])

        def squash_h(jh, r0=False, last=False):
            # v = s*inv_scale * g/(1+g) / sqrt(g+eps), g = ||s*inv_scale||^2
            # rsqrt via exp(-0.5*ln(g+eps)) -- same act table as exp.
            jsl = slice(jh * JH, (jh + 1) * JH)
            sc2 = (1.0 / (NJ * NJ)) if r0 else 1.0
            sc1 = (1.0 / NJ) if r0 else 1.0
            s_h = s_full[:, jsl, :]
            t = sc.tile([B, JH, DO], f32, tag="sqt", name="t")
            _stt(nc.vector, out=t, in0=s_h, in1=s_h, op1=AL.mult, scalar=sc2)
            g = sq[:, jsl]
            nc.vector.tensor_reduce(out=g, in_=t, axis=mybir.AxisListType.X,
                                    op=AL.add)
            lg = sc.tile([B, JH], f32, tag="lg", name="lg")
            nc.scalar.activation(out=lg, in_=g, func=AF.Ln, bias=eps_t)
            rs = sc.tile([B, JH], f32, tag="rs", name="rs")
            nc.scalar.activation(out=rs, in_=lg, func=AF.Exp, scale=-0.5)
            den = sc.tile([B, JH], f32, tag="den", name="den")
            nc.vector.scalar_tensor_tensor(out=den, in0=g, scalar=1.0,
                                           in1=rs, op0=AL.add, op1=AL.divide)
            f_h = fac[:, jsl]
            nc.vector.reciprocal(out=f_h, in_=den)
            out_t = v_out if last else v_jd
            _stt(nc.vector, out=out_t[:, jsl, :], in0=s_h,
                 in1=f_h.unsqueeze(2).broadcast_to([B, JH, DO]),
                 op1=AL.mult, scalar=sc1)

        def vT_h(jh):
            for jj in range(JH):
                j = jh * JH + jj
                ptv = ps.tile([128, B], f32, tag="tp", name="ptv",
                              padded_shape=[128, 512])
                nc.tensor.transpose(ptv[:DO, :], v_jd[:, j, :], ident_f)
                if jj % 2 == 0:
                    nc.vector.tensor_copy(out=v_T[:, j, :], in_=ptv[:DO, :])
                else:
                    nc.scalar.copy(out=v_T[:, j, :], in_=ptv[:DO, :])

        wd_f = wd_r.rearrange("d j i k -> d j (i k)")
        x_ik = x_bf.rearrange("b i k -> b (i k)")

        def c_update_h(jh, first):
            # m_j = sum_d v[b,j,d] W[j,:,d,:] (PE, bf16 psum out);
            # t_j = x * m_j; c_j += sum_k t_j (pair tree)
            jsl = slice(jh * JH, (jh + 1) * JH)
            t_flat = t_all.rearrange("b j i k -> b (j i k)")
            for jj in range(JH):
                j = jh * JH + jj
                pm = ps.tile([B, M_SL, 512], f32, tag="pm", name="pm")
                for sl in range(M_SL):
                    nc.tensor.matmul(
                        pm[:, sl, 0:M_FREE],
                        lhsT=v_T[:, j, :],
                        rhs=wd_f[:, j, M_FREE * sl:M_FREE * (sl + 1)],
                        start=True, stop=True,
                    )
                mode = T_MODE[j]
                t_out = t_flat[:, j * ni_l * DI:(j + 1) * ni_l * DI]
                if mode == "B":
                    m_bf = sc.tile([B, M_SL, M_FREE], bf16, tag="m_bf",
                                   name="m_bf")
                    nc.scalar.copy(out=m_bf, in_=pm[:, :, 0:M_FREE])
                    _stt(nc.vector, out=t_out,
                         in0=m_bf.rearrange("b s e -> b (s e)"),
                         in1=x_ik, op1=AL.mult)
                else:
                    eng = nc.gpsimd if mode == "D" else nc.vector
                    _stt(eng,
                         out=t_out.rearrange("b (s e) -> b s e", s=M_SL),
                         in0=pm[:, :, 0:M_FREE],
                         in1=x_ik.rearrange("b (s e) -> b s e", s=M_SL),
                         op1=AL.mult)
            # k pair-tree: 8 -> 4 -> 2 -> (+c)
            th = t_all[:, jsl, :, :]
            _stt(nc.vector, out=th[:, :, :, 0:4], in0=th[:, :, :, 0:4],
                 in1=th[:, :, :, 4:8], op1=AL.add)
            _stt(nc.gpsimd, out=th[:, :, :, 0:2], in0=th[:, :, :, 0:2],
                 in1=th[:, :, :, 2:4], op1=AL.add)
            cv = c_t[:, jsl, :]
            if first:
                _stt(nc.vector, out=cv, in0=th[:, :, :, 0],
                     in1=th[:, :, :, 1], op1=AL.add)
            else:
                _stt(nc.vector, out=cv, in0=cv, in1=th[:, :, :, 0], op1=AL.add)
                _stt(nc.vector, out=cv, in0=cv, in1=th[:, :, :, 1], op1=AL.add)
            # e = exp(c) for this half (feeds next round's softmax)
            nc.scalar.activation(out=e_bf[:, jsl, :], in_=cv, func=AF.Exp)

        def softmax_pre():
            # sigma = sum_j e (pair tree, all inputs stride-1), rin = 1/sigma
            e5 = e_bf
            _stt(nc.vector, out=sg1, in0=e5[:, 0:5, :], in1=e5[:, 5:10, :],
                 op1=AL.add)
            sg2 = sc.tile([B, 2, ni_l], f32, tag="sg2", name="sg2")
            _stt(nc.vector, out=sg2, in0=sg1[:, 0:2, :], in1=sg1[:, 2:4, :],
                 op1=AL.add)
            _stt(nc.vector, out=rin, in0=sg2[:, 0, :], in1=sg2[:, 1, :],
                 op1=AL.add)
            _stt(nc.vector, out=rin, in0=rin, in1=sg1[:, 4, :], op1=AL.add)
            nc.vector.reciprocal(out=rin, in_=rin)
            nc.vector.tensor_copy(out=rin_bf, in_=rin)
            # transposed 1/sigma: chunk0 plain, chunk1 k-replicated
            ptr = ps.tile([128, B], bf16, tag="tp", name="ptr", padded_shape=[128, 1024])
            nc.tensor.transpose(ptr, rin_bf[:, 0:N0], ident)
            nc.scalar.copy(out=rT0, in_=ptr)
            ptr = ps.tile([128, B], bf16, tag="tp", name="ptr", padded_shape=[128, 1024])
            nc.tensor.transpose(
                ptr,
                rin_bf[:, N0:ni_l].unsqueeze(2).broadcast_to([B, N1, DI]),
                ident)
            nc.vector.tensor_copy(out=rT1, in_=ptr)
            _stt(nc.vector, out=xs0, in0=x_P0,
                 in1=rT0.unsqueeze(1).broadcast_to([N0, DI, B]), op1=AL.mult)
            _stt(nc.vector, out=xs1, in0=x_P1, in1=rT1, op1=AL.mult)

        def s_half(jh):
            # returns psum tile with s partials for j in this half
            ps_h = ps.tile([B, JH, DO], f32, tag="pm", name="ps_h")
            for jj in range(JH):
                j = jh * JH + jj
                # eT chunk0: [128, B]; chunk1 k-replicated: [(i k), B]
                pe0 = ps.tile([128, B], bf16, tag="tp", name="pe0", padded_shape=[128, 1024])
                nc.tensor.transpose(pe0, e_bf[:, j, 0:N0], ident)
                pe1 = ps.tile([128, B], bf16, tag="tp", name="pe1", padded_shape=[128, 1024])
                nc.tensor.transpose(
                    pe1,
                    e_bf[:, j, N0:ni_l].unsqueeze(2).broadcast_to([B, N1, DI]),
                    ident)
                y_eng = nc.gpsimd if j in Y_POOL_J else nc.vector
                _stt(y_eng, out=y0, in0=xs0,
                     in1=pe0.unsqueeze(1).broadcast_to([N0, DI, B]),
                     op1=AL.mult)
                _stt(nc.vector, out=y1, in0=xs1, in1=pe1, op1=AL.mult)
                nc.tensor.matmul(ps_h[:, jj, :], lhsT=y1, rhs=w1[:, j, :],
                                 start=True, stop=False)
                for k in range(DI):
                    nc.tensor.matmul(
                        ps_h[:, jj, :],
                        lhsT=y0[:, k, :],
                        rhs=w0[:, j, :, k],
                        start=False, stop=(k == DI - 1),
                    )
            return ps_h

        # ---------------- r0: s0 = (1/NJ) * sum_ik x W  (scale in squash) ---
        ps0 = ps.tile([B, NJ, DO], f32, tag="pm", name="ps0")
        nc.tensor.matmul(ps0.rearrange("b j d -> b (j d)"), lhsT=x_P1,
                         rhs=w1.rearrange("p j d -> p (j d)"),
                         start=True, stop=False)
        for k in range(DI):
            nc.tensor.matmul(
                ps0.rearrange("b j d -> b (j d)"),
                lhsT=x_P0[:, k, :],
                rhs=w0[:, :, :, k].rearrange("i j d -> i (j d)"),
                start=False, stop=(k == DI - 1),
            )
        for jh in range(2):
            allreduce_h(jh, ps0[:, jh * JH:(jh + 1) * JH, :])
        for jh in range(2):
            squash_h(jh, r0=True)
            vT_h(jh)
            c_update_h(jh, first=True)

        # ---------------- routing rounds ----------------
        for r in range(1, ROUTINGS):
            last = (r == ROUTINGS - 1)
            softmax_pre()
            for jh in range(2):
                ps_h = s_half(jh)
                allreduce_h(jh, ps_h)
            for jh in range(2):
                squash_h(jh, last=last)
                if not last:
                    vT_h(jh)
                    c_update_h(jh, first=False)

        nc.sync.dma_start(out=out_d.ap(), in_=v_out)


_NC_CACHE = {}


def kernel(inputs: np.ndarray, W: np.ndarray) -> np.ndarray:
    import ml_dtypes
    bf = ml_dtypes.bfloat16
    n_cores = 8
    ni_l = NI // n_cores
    if "nc" not in _NC_CACHE:
        _NC_CACHE["nc"] = build_kernel(n_cores=n_cores, debug=False)
    nc = _NC_CACHE["nc"]
    in_maps = []
    for r in range(n_cores):
        sl = slice(ni_l * r, ni_l * (r + 1))
        Wl = np.ascontiguousarray(W[:, sl], dtype=np.float32)  # [NJ,ni_l,DO,DI]
        w0 = np.ascontiguousarray(
            Wl[:, 0:N0].transpose(1, 0, 2, 3)).astype(bf)      # [N0,NJ,DO,DI]
        w1 = np.ascontiguousarray(
            Wl[:, N0:ni_l].transpose(1, 3, 0, 2).reshape(
                N1 * DI, NJ, DO)).astype(bf)                   # [(i k),NJ,DO]
        wdl = np.ascontiguousarray(Wl.transpose(2, 0, 1, 3))   # [DO,NJ,ni_l,DI]
        in_maps.append({
            "x": np.ascontiguousarray(inputs[:, sl, :]).astype(bf),
            "w0": w0,
            "w1": w1,
            "wd": wdl,
        })
    res = run_bass_kernel_spmd(nc, in_maps, core_ids=list(range(n_cores)))
    return res.results[0]["out"]


# revision 12
# speedup vs baseline: 1.1605x; 1.1605x over previous
"""Trainium2 Bass kernel for nn_CapsuleLayer (dynamic routing capsule layer).

Sharding: the 1152 input capsules (i) are split across 8 cores (144 each);
the full batch B=128 lives on SBUF partitions. Routing state (c, p) stays
local to each core's i-shard; per-iteration s partial sums are combined with
small AllReduces ([128,80] f32, one per j-half so the collective latency
hides under the other half's compute). u_hat is never materialized:
  s[b,j,d]       = sum_{i,k} (exp(c)/sigma)[b,j,i] x[b,i,k] W[j,i,d,k]  (PE)
  c_delta[b,j,i] = sum_k x[b,i,k] m[b,j,i,k],   m = sum_d v[b,j,d] W[j,i,d,k]
Host pre-lays W out in three device layouts (i-major bf16 for s, (i*k)-packed
bf16 for the 16-capsule remainder chunk, d-major f32 for m) so the device
does no staging casts. All activation funcs (exp, ln for rsqrt) come from
one table set, so LoadActFuncSet is emitted once.
"""

import sys

if "/opt/trn_rl_repo" not in sys.path:
    sys.path.insert(0, "/opt/trn_rl_repo")

import contextlib

import numpy as np

import concourse.bass as bass  # noqa: F401
import concourse.tile as tile
from concourse import bacc, mybir
from concourse.bass_utils import run_bass_kernel_spmd
from concourse.masks import make_identity


def _steer_act_tables():
    # Compile-time table-choice steering: strip the funcs we use (exp/ln/
    # copy/identity) from every set that precedes natural_log_exp_and_others
    # so the selector lands all activations on that one set -> a single
    # LoadActFuncSet for the whole kernel. Indices of the sets are preserved,
    # so the emitted act_func_set_id still matches act_info.json.
    from concourse import bacc as _bacc_mod
    import concourse.hw_specs as _hws
    if getattr(_steer_act_tables, "_done", False):
        return
    _orig = _hws.get_activation_tables

    def _patched(arch):
        tabs = _orig(arch)
        target = "natural_log_exp_and_others"
        if target not in tabs:
            return tabs
        keep = tabs[target]
        out = {}
        for name, funcs in tabs.items():
            out[name] = funcs if name == target else (funcs - keep)
        return out

    _bacc_mod.get_activation_tables = _patched
    _steer_act_tables._done = True

f32 = mybir.dt.float32
f32r = mybir.dt.float32r
bf16 = mybir.dt.bfloat16
AL = mybir.AluOpType
AF = mybir.ActivationFunctionType

B = 128          # batch (on partitions)
NJ = 10          # output capsules
DO = 16          # output capsule dim
DI = 8           # input capsule dim
NI = 1152        # input capsules (global)
ROUTINGS = 3
EPS = 1e-7

N0 = 128         # chunk0: i = 0..127, i on partitions
N1 = 16          # chunk1: i = 128..143, (i,k) packed on partitions (16*8=128)
M_SL = 3         # m-matmul free-dim slices (3 x 384 = 1152)
M_FREE = (N0 + N1) * DI // M_SL

# engine assignment knobs (tuned against TimelineSim)
# t-multiply mode per j: A = DVE direct from PSUM, B = Act evac + DVE mult,
# D = Pool direct from PSUM
T_MODE = "BBDAB" * 2
Y_POOL_J = (3, 6, 9)         # j's whose chunk0 y-multiply runs on Pool


def _stt(eng, out, in0, in1, op1, scalar=None, op0=None):
    if scalar is not None and scalar != 1.0 or op0 is not None and op0 != AL.mult:
        eng.scalar_tensor_tensor(out=out, in0=in0, scalar=scalar or 1.0,
                                 in1=in1, op0=op0 or AL.mult, op1=op1)
    else:
        eng.tensor_tensor(out=out, in0=in0, in1=in1, op=op1)


def build_kernel(n_cores=8, debug=False, repeat=1, single=False, ablate=()):
    ni_l = NI // n_cores
    assert ni_l == N0 + N1

    _steer_act_tables()
    nc = bacc.Bacc("TRN2", target_bir_lowering=False, debug=False,
                   num_devices=1 if single else n_cores)
    # host-prepped layouts (see kernel() below)
    x_d = nc.dram_tensor("x", [B, ni_l, DI], bf16, kind="ExternalInput")
    w0_d = nc.dram_tensor("w0", [N0, NJ, DO, DI], bf16, kind="ExternalInput")
    w1_d = nc.dram_tensor("w1", [N1 * DI, NJ, DO], bf16, kind="ExternalInput")
    wd_d = nc.dram_tensor("wd", [DO, NJ, ni_l, DI], f32, kind="ExternalInput")
    out_d = nc.dram_tensor("out", [B, NJ, DO], f32, kind="ExternalOutput")

    with tile.TileContext(nc) as tc:
        for _rep in range(repeat):
            _body(nc, tc, x_d, w0_d, w1_d, wd_d, out_d, ni_l, n_cores, single)
    nc.compile()
    return nc


def _body(nc, tc, x_d, w0_d, w1_d, wd_d, out_d, ni_l, n_cores, single=False):
    ctx = contextlib.ExitStack()
    JH = NJ // 2
    with ctx:
        sb = ctx.enter_context(tc.tile_pool(name="sb", bufs=1))
        sc = ctx.enter_context(tc.tile_pool(name="scratch", bufs=2))
        ps = ctx.enter_context(tc.tile_pool(name="ps", bufs=2, space="PSUM"))
        dram = ctx.enter_context(tc.tile_pool(name="dram", bufs=2, space="DRAM"))

        # ---------------- loads (no staging: host sent final layouts) -------
        x_bf = sb.tile([B, ni_l, DI], bf16)
        nc.sync.dma_start(out=x_bf.rearrange("b i k -> b (i k)"),
                          in_=x_d.ap().rearrange("b i k -> b (i k)"))
        w0 = sb.tile([N0, NJ, DO, DI], bf16)
        nc.sync.dma_start(out=w0.rearrange("i j d k -> i (j d k)"),
                          in_=w0_d.ap().rearrange("i j d k -> i (j d k)"))
        w1 = sb.tile([N1 * DI, NJ, DO], bf16)
        nc.sync.dma_start(out=w1.rearrange("p j d -> p (j d)"),
                          in_=w1_d.ap().rearrange("p j d -> p (j d)"))
        wd = sb.tile([DO, NJ, ni_l, DI], f32)
        nc.sync.dma_start(out=wd.rearrange("d j i k -> d (j i k)"),
                          in_=wd_d.ap().rearrange("d j i k -> d (j i k)"))
        wd_r = wd.bitcast(f32r)

        ident = sb.tile([128, 128], bf16)
        make_identity(nc, ident)
        ident_f = sb.tile([128, 128], f32)
        make_identity(nc, ident_f)

        # x_P0: [i0, k, b]; x_P1: [(i1 k), b]  (PE transposes via PSUM)
        x_P0 = sb.tile([N0, DI, B], bf16)
        x_P1 = sb.tile([N1 * DI, B], bf16)
        for k in range(DI):
            pt = ps.tile([128, B], bf16, tag="tp", name="pt", padded_shape=[128, 1024])
            nc.tensor.transpose(pt, x_bf[:, 0:N0, k], ident)
            eng = nc.vector if k % 2 == 0 else nc.scalar
            if eng is nc.scalar:
                eng.copy(out=x_P0[:, k, :], in_=pt)
            else:
                eng.tensor_copy(out=x_P0[:, k, :], in_=pt)
        pt = ps.tile([128, B], bf16, tag="tp", name="pt", padded_shape=[128, 1024])
        nc.tensor.transpose(
            pt, x_bf[:, N0:ni_l, :].rearrange("b i k -> b (i k)"), ident)
        nc.vector.tensor_copy(out=x_P1, in_=pt)

        # persistent state
        c_t = sb.tile([B, NJ, ni_l], f32)        # routing logits
        s_full = sb.tile([B, NJ, DO], f32)       # all-reduced s (j,d)
        v_jd = sb.tile([B, NJ, DO], f32)         # squashed v (j,d)
        v_T = sb.tile([DO, NJ, B], f32r)         # v transposed [d, j, b]
        e_bf = sb.tile([B, NJ, ni_l], bf16)      # exp(c)
        sg1 = sb.tile([B, 5, ni_l], f32)         # sigma tree scratch
        rin = sb.tile([B, ni_l], f32)            # 1/sigma
        rin_bf = sb.tile([B, ni_l], bf16)
        rT0 = sb.tile([N0, B], bf16)
        rT1 = sb.tile([N1 * DI, B], bf16)
        xs0 = sb.tile([N0, DI, B], bf16)         # x * (1/sigma), chunk0
        xs1 = sb.tile([N1 * DI, B], bf16)
        t_all = sb.tile([B, NJ, ni_l, DI], bf16)  # m * x scratch
        sq = sb.tile([B, NJ], f32)
        fac = sb.tile([B, NJ], f32)
        v_out = sb.tile([B, NJ, DO], f32)
        eps_t = sb.tile([B, 1], f32)
        nc.vector.memset(eps_t, EPS)

        ar_in = [dram.tile([B, JH * DO], f32, tag=f"ai{h}", name=f"ai{h}") for h in range(2)]
        ar_out = [dram.tile([B, JH * DO], f32, tag=f"ao{h}", name=f"ao{h}") for h in range(2)]
        arf_in = dram.tile([B, NJ * DO], f32, tag="afi", name="afi")
        arf_out = dram.tile([B, NJ * DO], f32, tag="afo", name="afo")
        s_part_f = sb.tile([B, NJ * DO], f32)

        s_part = [sb.tile([B, JH * DO], f32, tag=f"sp{h}", name=f"sp{h}")
                  for h in range(2)]

        def allreduce_h(jh, ps_h):
            # PSUM -> SBUF -> DRAM -> (collective) -> SBUF half of s_full
            nc.scalar.copy(out=s_part[jh],
                           in_=ps_h.rearrange("b j d -> b (j d)"))
            nc.sync.dma_start(out=ar_in[jh], in_=s_part[jh])
            if single:
                nc.sync.dma_start(out=ar_out[jh], in_=ar_in[jh])
            else:
                nc.gpsimd.collective_compute(
                    "AllReduce", AL.add,
                    ins=[ar_in[jh].opt()], outs=[ar_out[jh].opt()],
                    replica_groups=[list(range(n_cores))],
                )
            jsl = slice(jh * JH, (jh + 1) * JH)
            nc.sync.dma_start(
                out=s_full[:, jsl, :].rearrange("b j d -> b (j d)"),
                in_=ar_out[jh])

        def squash_h(jh, r0=False, last=False):
            # v = s*inv_scale * g/(1+g) / sqrt(g+eps), g = ||s*inv_scale||^2
            # rsqrt via exp(-0.5*ln(g+eps)) -- same act table as exp.
            jsl = slice(jh * JH, (jh + 1) * JH)
            sc2 = (1.0 / (NJ * NJ)) if r0 else 1.0
            sc1 = (1.0 / NJ) if r0 else 1.0
            s_h = s_full[:, jsl, :]
            t = sc.tile([B, JH, DO], f32, tag="sqt", name="t")
            _stt(nc.vector, out=t, in0=s_h, in1=s_h, op1=AL.mult, scalar=sc2)
            g = sq[:, jsl]
            nc.vector.tensor_reduce(out=g, in_=t, axis=mybir.AxisListType.X,
                                    op=AL.add)
            lg = sc.tile([B, JH], f32, tag="lg", name="lg")
            nc.scalar.activation(out=lg, in_=g, func=AF.Ln, bias=eps_t)
            rs = sc.tile([B, JH], f32, tag="rs", name="rs")
            nc.scalar.activation(out=rs, in_=lg, func=AF.Exp, scale=-0.5)
            den = sc.tile([B, JH], f32, tag="den", name="den")
            nc.vector.scalar_tensor_tensor(out=den, in0=g, scalar=1.0,
                                           in1=rs, op0=AL.add, op1=AL.divide)
            nc.vector.reciprocal(out=den, in_=den)
            f_h = fac[:, jsl]
            nc.vector.tensor_tensor(out=f_h, in0=g, in1=den, op=AL.mult)
            out_t = v_out if last else v_jd
            _stt(nc.vector, out=out_t[:, jsl, :], in0=s_h,
                 in1=f_h.unsqueeze(2).broadcast_to([B, JH, DO]),
                 op1=AL.mult, scalar=sc1)

        def vT_h(jh):
            for jj in range(JH):
                j = jh * JH + jj
                ptv = ps.tile([128, B], f32, tag="tp", name="ptv",
                              padded_shape=[128, 512])
                nc.tensor.transpose(ptv[:DO, :], v_jd[:, j, :], ident_f)
                if jj % 2 == 0:
                    nc.vector.tensor_copy(out=v_T[:, j, :], in_=ptv[:DO, :])
                else:
                    nc.scalar.copy(out=v_T[:, j, :], in_=ptv[:DO, :])

        wd_f = wd_r.rearrange("d j i k -> d j (i k)")
        x_ik = x_bf.rearrange("b i k -> b (i k)")

        def c_update_h(jh, first):
            # m_j = sum_d v[b,j,d] W[j,:,d,:] (PE, bf16 psum out);
            # t_j = x * m_j; c_j += sum_k t_j (pair tree)
            jsl = slice(jh * JH, (jh + 1) * JH)
            t_flat = t_all.rearrange("b j i k -> b (j i k)")
            for jj in range(JH):
                j = jh * JH + jj
                pm = ps.tile([B, M_SL, 512], f32, tag="pm", name="pm")
                for sl in range(M_SL):
                    nc.tensor.matmul(
                        pm[:, sl, 0:M_FREE],
                        lhsT=v_T[:, j, :],
                        rhs=wd_f[:, j, M_FREE * sl:M_FREE * (sl + 1)],
                        start=True, stop=True,
                    )
                mode = T_MODE[j]
                t_out = t_flat[:, j * ni_l * DI:(j + 1) * ni_l * DI]
                if mode == "B":
                    m_bf = sc.tile([B, M_SL, M_FREE], bf16, tag="m_bf",
                                   name="m_bf")
                    nc.scalar.copy(out=m_bf, in_=pm[:, :, 0:M_FREE])
                    _stt(nc.vector, out=t_out,
                         in0=m_bf.rearrange("b s e -> b (s e)"),
                         in1=x_ik, op1=AL.mult)
                else:
                    eng = nc.gpsimd if mode == "D" else nc.vector
                    _stt(eng,
                         out=t_out.rearrange("b (s e) -> b s e", s=M_SL),
                         in0=pm[:, :, 0:M_FREE],
                         in1=x_ik.rearrange("b (s e) -> b s e", s=M_SL),
                         op1=AL.mult)
            # k pair-tree: 8 -> 4 -> 2 -> (+c)
            th = t_all[:, jsl, :, :]
            _stt(nc.vector, out=th[:, :, :, 0:4], in0=th[:, :, :, 0:4],
                 in1=th[:, :, :, 4:8], op1=AL.add)
            _stt(nc.vector, out=th[:, :, :, 0:2], in0=th[:, :, :, 0:2],
                 in1=th[:, :, :, 2:4], op1=AL.add)
            cv = c_t[:, jsl, :]
            if first:
                _stt(nc.vector, out=cv, in0=th[:, :, :, 0],
                     in1=th[:, :, :, 1], op1=AL.add)
            else:
                _stt(nc.vector, out=cv, in0=cv, in1=th[:, :, :, 0], op1=AL.add)
                _stt(nc.vector, out=cv, in0=cv, in1=th[:, :, :, 1], op1=AL.add)
            # e = exp(c) for this half (feeds next round's softmax)
            nc.scalar.activation(out=e_bf[:, jsl, :], in_=cv, func=AF.Exp)

        def softmax_pre():
            # sigma = sum_j e (pair tree, all inputs stride-1), rin = 1/sigma
            e5 = e_bf
            _stt(nc.vector, out=sg1, in0=e5[:, 0:5, :], in1=e5[:, 5:10, :],
                 op1=AL.add)
            sg2 = sc.tile([B, 2, ni_l], f32, tag="sg2", name="sg2")
            _stt(nc.vector, out=sg2, in0=sg1[:, 0:2, :], in1=sg1[:, 2:4, :],
                 op1=AL.add)
            _stt(nc.vector, out=rin, in0=sg2[:, 0, :], in1=sg2[:, 1, :],
                 op1=AL.add)
            _stt(nc.vector, out=rin, in0=rin, in1=sg1[:, 4, :], op1=AL.add)
            nc.vector.reciprocal(out=rin, in_=rin)
            nc.vector.tensor_copy(out=rin_bf, in_=rin)
            # transposed 1/sigma: chunk0 plain, chunk1 k-replicated
            ptr = ps.tile([128, B], bf16, tag="tp", name="ptr", padded_shape=[128, 1024])
            nc.tensor.transpose(ptr, rin_bf[:, 0:N0], ident)
            nc.scalar.copy(out=rT0, in_=ptr)
            ptr = ps.tile([128, B], bf16, tag="tp", name="ptr", padded_shape=[128, 1024])
            nc.tensor.transpose(
                ptr,
                rin_bf[:, N0:ni_l].unsqueeze(2).broadcast_to([B, N1, DI]),
                ident)
            nc.vector.tensor_copy(out=rT1, in_=ptr)
            _stt(nc.vector, out=xs0, in0=x_P0,
                 in1=rT0.unsqueeze(1).broadcast_to([N0, DI, B]), op1=AL.mult)
            _stt(nc.vector, out=xs1, in0=x_P1, in1=rT1, op1=AL.mult)

        def s_half(jh):
            # returns psum tile with s partials for j in this half
            ps_h = ps.tile([B, JH, DO], f32, tag="pm", name="ps_h")
            for jj in range(JH):
                j = jh * JH + jj
                # eT chunk0: [128, B]; chunk1 k-replicated: [(i k), B]
                pe0 = ps.tile([128, B], bf16, tag="tp", name="pe0", padded_shape=[128, 1024])
                nc.tensor.transpose(pe0, e_bf[:, j, 0:N0], ident)
                pe1 = ps.tile([128, B], bf16, tag="tp", name="pe1", padded_shape=[128, 1024])
                nc.tensor.transpose(
                    pe1,
                    e_bf[:, j, N0:ni_l].unsqueeze(2).broadcast_to([B, N1, DI]),
                    ident)
                y0 = sc.tile([N0, DI, B], bf16, tag="y0", name="y0")
                y1 = sc.tile([N1 * DI, B], bf16, tag="y1", name="y1")
                y_eng = nc.gpsimd if j in Y_POOL_J else nc.vector
                _stt(y_eng, out=y0, in0=xs0,
                     in1=pe0.unsqueeze(1).broadcast_to([N0, DI, B]),
                     op1=AL.mult)
                _stt(nc.vector, out=y1, in0=xs1, in1=pe1, op1=AL.mult)
                nc.tensor.matmul(ps_h[:, jj, :], lhsT=y1, rhs=w1[:, j, :],
                                 start=True, stop=False)
                for k in range(DI):
                    nc.tensor.matmul(
                        ps_h[:, jj, :],
                        lhsT=y0[:, k, :],
                        rhs=w0[:, j, :, k],
                        start=False, stop=(k == DI - 1),
                    )
            return ps_h

        # ---------------- r0: s0 = (1/NJ) * sum_ik x W  (scale in squash) ---
        ps0 = ps.tile([B, NJ, DO], f32, tag="pm", name="ps0")
        nc.tensor.matmul(ps0.rearrange("b j d -> b (j d)"), lhsT=x_P1,
                         rhs=w1.rearrange("p j d -> p (j d)"),
                         start=True, stop=False)
        for k in range(DI):
            nc.tensor.matmul(
                ps0.rearrange("b j d -> b (j d)"),
                lhsT=x_P0[:, k, :],
                rhs=w0[:, :, :, k].rearrange("i j d -> i (j d)"),
                start=False, stop=(k == DI - 1),
            )
        nc.scalar.copy(out=s_part_f, in_=ps0.rearrange("b j d -> b (j d)"))
        nc.sync.dma_start(out=arf_in, in_=s_part_f)
        if single:
            nc.sync.dma_start(out=arf_out, in_=arf_in)
        else:
            nc.gpsimd.collective_compute(
                "AllReduce", AL.add,
                ins=[arf_in.opt()], outs=[arf_out.opt()],
                replica_groups=[list(range(n_cores))],
            )
        nc.sync.dma_start(out=s_full.rearrange("b j d -> b (j d)"), in_=arf_out)
        for jh in range(2):
            squash_h(jh, r0=True)
            vT_h(jh)
            c_update_h(jh, first=True)

        # ---------------- routing rounds ----------------
        for r in range(1, ROUTINGS):
            last = (r == ROUTINGS - 1)
            softmax_pre()
            for jh in range(2):
                ps_h = s_half(jh)
                allreduce_h(jh, ps_h)
            for jh in range(2):
                squash_h(jh, last=last)
                if not last:
                    vT_h(jh)
                    c_update_h(jh, first=False)

        nc.sync.dma_start(out=out_d.ap(), in_=v_out)


_NC_CACHE = {}


def kernel(inputs: np.ndarray, W: np.ndarray) -> np.ndarray:
    import ml_dtypes
    bf = ml_dtypes.bfloat16
    n_cores = 8
    ni_l = NI // n_cores
    if "nc" not in _NC_CACHE:
        _NC_CACHE["nc"] = build_kernel(n_cores=n_cores, debug=False)
    nc = _NC_CACHE["nc"]
    in_maps = []
    for r in range(n_cores):
        sl = slice(ni_l * r, ni_l * (r + 1))
        Wl = np.ascontiguousarray(W[:, sl], dtype=np.float32)  # [NJ,ni_l,DO,DI]
        w0 = np.ascontiguousarray(
            Wl[:, 0:N0].transpose(1, 0, 2, 3)).astype(bf)      # [N0,NJ,DO,DI]
        w1 = np.ascontiguousarray(
            Wl[:, N0:ni_l].transpose(1, 3, 0, 2).reshape(
                N1 * DI, NJ, DO)).astype(bf)                   # [(i k),NJ,DO]
        wdl = np.ascontiguousarray(Wl.transpose(2, 0, 1, 3))   # [DO,NJ,ni_l,DI]
        in_maps.append({
            "x": np.ascontiguousarray(inputs[:, sl, :]).astype(bf),
            "w0": w0,
            "w1": w1,
            "wd": wdl,
        })
    res = run_bass_kernel_spmd(nc, in_maps, core_ids=list(range(n_cores)))
    return res.results[0]["out"]


# revision 17
# speedup vs baseline: 1.1851x; 1.0212x over previous
"""Trainium2 Bass kernel for nn_CapsuleLayer (dynamic routing capsule layer).

Sharding: the 1152 input capsules (i) are split across 8 cores (144 each);
the full batch B=128 lives on SBUF partitions. Routing state (c, p) stays
local to each core's i-shard; per-iteration s partial sums are combined with
small AllReduces ([128,80] f32, one per j-half so the collective latency
hides under the other half's compute). u_hat is never materialized:
  s[b,j,d]       = sum_{i,k} (exp(c)/sigma)[b,j,i] x[b,i,k] W[j,i,d,k]  (PE)
  c_delta[b,j,i] = sum_k x[b,i,k] m[b,j,i,k],   m = sum_d v[b,j,d] W[j,i,d,k]
Host pre-lays W out in three device layouts (i-major bf16 for s, (i*k)-packed
bf16 for the 16-capsule remainder chunk, d-major f32 for m) so the device
does no staging casts. All activation funcs (exp, ln for rsqrt) come from
one table set, so LoadActFuncSet is emitted once.
"""

import sys

if "/opt/trn_rl_repo" not in sys.path:
    sys.path.insert(0, "/opt/trn_rl_repo")

import contextlib

import numpy as np

import concourse.bass as bass  # noqa: F401
import concourse.tile as tile
from concourse import bacc, mybir
from concourse.bass_utils import run_bass_kernel_spmd
from concourse.masks import make_identity


def _steer_act_tables():
    # Compile-time table-choice steering: strip the funcs we use (exp/ln/
    # copy/identity) from every set that precedes natural_log_exp_and_others
    # so the selector lands all activations on that one set -> a single
    # LoadActFuncSet for the whole kernel. Indices of the sets are preserved,
    # so the emitted act_func_set_id still matches act_info.json.
    from concourse import bacc as _bacc_mod
    import concourse.hw_specs as _hws
    if getattr(_steer_act_tables, "_done", False):
        return
    _orig = _hws.get_activation_tables

    def _patched(arch):
        tabs = _orig(arch)
        target = "natural_log_exp_and_others"
        if target not in tabs:
            return tabs
        keep = tabs[target]
        out = {}
        for name, funcs in tabs.items():
            out[name] = funcs if name == target else (funcs - keep)
        return out

    _bacc_mod.get_activation_tables = _patched
    _steer_act_tables._done = True

f32 = mybir.dt.float32
f32r = mybir.dt.float32r
bf16 = mybir.dt.bfloat16
AL = mybir.AluOpType
AF = mybir.ActivationFunctionType

B = 128          # batch (on partitions)
NJ = 10          # output capsules
DO = 16          # output capsule dim
DI = 8           # input capsule dim
NI = 1152        # input capsules (global)
ROUTINGS = 3
EPS = 1e-7

N0 = 128         # chunk0: i = 0..127, i on partitions
N1 = 16          # chunk1: i = 128..143, (i,k) packed on partitions (16*8=128)
M_SL = 3         # m-matmul free-dim slices (3 x 384 = 1152)
M_FREE = (N0 + N1) * DI // M_SL

# engine assignment knobs (tuned against TimelineSim)
# t-multiply mode per j: A = DVE direct from PSUM, B = Act evac + DVE mult,
# D = Pool direct from PSUM
T_MODE = "ABBCB" * 2
Y_POOL_J = (2, 5, 8)         # j's whose chunk0 y-multiply runs on Pool (from an Act-evac'd SBUF copy; GPSIMD cannot read PSUM)


def _stt(eng, out, in0, in1, op1, scalar=None, op0=None):
    if scalar is not None and scalar != 1.0 or op0 is not None and op0 != AL.mult:
        eng.scalar_tensor_tensor(out=out, in0=in0, scalar=scalar or 1.0,
                                 in1=in1, op0=op0 or AL.mult, op1=op1)
    else:
        eng.tensor_tensor(out=out, in0=in0, in1=in1, op=op1)


def build_kernel(n_cores=8, debug=False, repeat=1, single=False, ablate=()):
    ni_l = NI // n_cores
    assert ni_l == N0 + N1

    _steer_act_tables()
    nc = bacc.Bacc("TRN2", target_bir_lowering=False, debug=False,
                   num_devices=1 if single else n_cores)
    # host-prepped layouts (see kernel() below)
    x_d = nc.dram_tensor("x", [B, ni_l, DI], bf16, kind="ExternalInput")
    w0_d = nc.dram_tensor("w0", [N0, NJ, DO, DI], bf16, kind="ExternalInput")
    w1_d = nc.dram_tensor("w1", [N1 * DI, NJ, DO], bf16, kind="ExternalInput")
    wd_d = nc.dram_tensor("wd", [DO, NJ, ni_l, DI], f32, kind="ExternalInput")
    out_d = nc.dram_tensor("out", [B, NJ, DO], f32, kind="ExternalOutput")

    with tile.TileContext(nc) as tc:
        for _rep in range(repeat):
            _body(nc, tc, x_d, w0_d, w1_d, wd_d, out_d, ni_l, n_cores, single)
    nc.compile()
    return nc


def _body(nc, tc, x_d, w0_d, w1_d, wd_d, out_d, ni_l, n_cores, single=False):
    ctx = contextlib.ExitStack()
    JH = NJ // 2
    with ctx:
        sb = ctx.enter_context(tc.tile_pool(name="sb", bufs=1))
        sc = ctx.enter_context(tc.tile_pool(name="scratch", bufs=2))
        ps = ctx.enter_context(tc.tile_pool(name="ps", bufs=2, space="PSUM"))
        dram = ctx.enter_context(tc.tile_pool(name="dram", bufs=2, space="DRAM"))

        # ---------------- loads (no staging: host sent final layouts) -------
        x_bf = sb.tile([B, ni_l, DI], bf16)
        nc.sync.dma_start(out=x_bf.rearrange("b i k -> b (i k)"),
                          in_=x_d.ap().rearrange("b i k -> b (i k)"))
        w0 = sb.tile([N0, NJ, DO, DI], bf16)
        nc.sync.dma_start(out=w0.rearrange("i j d k -> i (j d k)"),
                          in_=w0_d.ap().rearrange("i j d k -> i (j d k)"))
        w1 = sb.tile([N1 * DI, NJ, DO], bf16)
        nc.sync.dma_start(out=w1.rearrange("p j d -> p (j d)"),
                          in_=w1_d.ap().rearrange("p j d -> p (j d)"))
        wd_r = sb.tile([DO, NJ, ni_l, DI], f32r, name="wd_r")
        nc.sync.dma_start(
            out=wd_r.rearrange("d j i k -> d (j i k)"),
            in_=wd_d.ap().rearrange("d j i k -> d (j i k)").bitcast(f32r))

        ident = sb.tile([128, 128], bf16)
        make_identity(nc, ident)
        ident_f = sb.tile([128, 128], f32)
        make_identity(nc, ident_f)

        # x_P0: [i0, k, b]; x_P1: [(i1 k), b]  (PE transposes via PSUM)
        x_P0 = sb.tile([N0, DI, B], bf16)
        x_P1 = sb.tile([N1 * DI, B], bf16)
        for k in range(DI):
            pt = ps.tile([128, B], bf16, tag="tp", name="pt", padded_shape=[128, 1024])
            nc.tensor.transpose(pt, x_bf[:, 0:N0, k], ident)
            eng = nc.vector if k % 2 == 0 else nc.scalar
            if eng is nc.scalar:
                eng.copy(out=x_P0[:, k, :], in_=pt)
            else:
                eng.tensor_copy(out=x_P0[:, k, :], in_=pt)
        pt = ps.tile([128, B], bf16, tag="tp", name="pt", padded_shape=[128, 1024])
        nc.tensor.transpose(
            pt, x_bf[:, N0:ni_l, :].rearrange("b i k -> b (i k)"), ident)
        nc.vector.tensor_copy(out=x_P1, in_=pt)

        # persistent state
        c_t = sb.tile([B, NJ, ni_l], f32)        # routing logits
        s_full = sb.tile([B, NJ, DO], f32)       # all-reduced s (j,d)
        v_jd = sb.tile([B, NJ, DO], f32)         # squashed v (j,d)
        v_T = sb.tile([DO, NJ, B], f32r)         # v transposed [d, j, b]
        e_bf = sb.tile([B, NJ, ni_l], bf16)      # exp(c)
        sg1 = sb.tile([B, 5, ni_l], f32)         # sigma tree scratch
        rin = sb.tile([B, ni_l], f32)            # 1/sigma
        rin_bf = sb.tile([B, ni_l], bf16)
        rT0 = sb.tile([N0, B], bf16)
        rT1 = sb.tile([N1 * DI, B], bf16)
        xs0 = sb.tile([N0, DI, B], bf16)         # x * (1/sigma), chunk0
        xs1 = sb.tile([N1 * DI, B], bf16)
        t_all = sb.tile([B, NJ, ni_l, DI], bf16)  # m * x scratch
        sq = sb.tile([B, NJ], f32)
        fac = sb.tile([B, NJ], f32)
        v_out = sb.tile([B, NJ, DO], f32)
        eps_t = sb.tile([B, 1], f32)
        nc.vector.memset(eps_t, EPS)

        ar_in = [dram.tile([B, JH * DO], f32, tag=f"ai{h}", name=f"ai{h}") for h in range(2)]
        ar_out = [dram.tile([B, JH * DO], f32, tag=f"ao{h}", name=f"ao{h}") for h in range(2)]
        arf_in = dram.tile([B, NJ * DO], f32, tag="afi", name="afi")
        arf_out = dram.tile([B, NJ * DO], f32, tag="afo", name="afo")
        s_part_f = sb.tile([B, NJ * DO], f32)

        s_part = [sb.tile([B, JH * DO], f32, tag=f"sp{h}", name=f"sp{h}")
                  for h in range(2)]

        def allreduce_h(jh, ps_h):
            # PSUM -> SBUF -> DRAM -> (collective) -> SBUF half of s_full
            nc.scalar.copy(out=s_part[jh],
                           in_=ps_h.rearrange("b j d -> b (j d)"))
            nc.sync.dma_start(out=ar_in[jh], in_=s_part[jh])
            if single:
                nc.sync.dma_start(out=ar_out[jh], in_=ar_in[jh])
            else:
                nc.gpsimd.collective_compute(
                    "AllReduce", AL.add,
                    ins=[ar_in[jh].opt()], outs=[ar_out[jh].opt()],
                    replica_groups=[list(range(n_cores))],
                )
            jsl = slice(jh * JH, (jh + 1) * JH)
            nc.sync.dma_start(
                out=s_full[:, jsl, :].rearrange("b j d -> b (j d)"),
                in_=ar_out[jh])

        def squash_h(jh, r0=False, last=False):
            # v = s*inv_scale * g/(1+g) / sqrt(g+eps), g = ||s*inv_scale||^2
            # rsqrt via exp(-0.5*ln(g+eps)) -- same act table as exp.
            jsl = slice(jh * JH, (jh + 1) * JH)
            sc2 = (1.0 / (NJ * NJ)) if r0 else 1.0
            sc1 = (1.0 / NJ) if r0 else 1.0
            s_h = s_full[:, jsl, :]
            t = sc.tile([B, JH, DO], f32, tag="sqt", name="t")
            _stt(nc.vector, out=t, in0=s_h, in1=s_h, op1=AL.mult, scalar=sc2)
            g = sq[:, jsl]
            nc.vector.tensor_reduce(out=g, in_=t, axis=mybir.AxisListType.X,
                                    op=AL.add)
            lg = sc.tile([B, JH], f32, tag="lg", name="lg")
            nc.scalar.activation(out=lg, in_=g, func=AF.Ln, bias=eps_t)
            rs = sc.tile([B, JH], f32, tag="rs", name="rs")
            nc.scalar.activation(out=rs, in_=lg, func=AF.Exp, scale=-0.5)
            den = sc.tile([B, JH], f32, tag="den", name="den")
            nc.vector.tensor_scalar(out=den, in0=g, scalar1=1.0, scalar2=None,
                                    op0=AL.add)
            nc.vector.reciprocal(out=den, in_=den)
            gr = sc.tile([B, JH], f32, tag="gr", name="gr")
            nc.vector.tensor_tensor(out=gr, in0=g, in1=rs, op=AL.mult)
            f_h = fac[:, jsl]
            nc.vector.tensor_tensor(out=f_h, in0=gr, in1=den, op=AL.mult)
            out_t = v_out if last else v_jd
            _stt(nc.vector, out=out_t[:, jsl, :], in0=s_h,
                 in1=f_h.unsqueeze(2).broadcast_to([B, JH, DO]),
                 op1=AL.mult, scalar=sc1)

        def vT_h(jh):
            for jj in range(JH):
                j = jh * JH + jj
                ptv = ps.tile([128, B], f32, tag="tp", name="ptv",
                              padded_shape=[128, 512])
                nc.tensor.transpose(ptv[:DO, :], v_jd[:, j, :], ident_f)
                if jj % 2 == 0:
                    nc.vector.tensor_copy(out=v_T[:, j, :], in_=ptv[:DO, :])
                else:
                    nc.scalar.copy(out=v_T[:, j, :], in_=ptv[:DO, :])

        wd_f = wd_r.rearrange("d j i k -> d j (i k)")
        x_ik = x_bf.rearrange("b i k -> b (i k)")

        def c_update_h(jh, first):
            # m_j = sum_d v[b,j,d] W[j,:,d,:] (PE, bf16 psum out);
            # t_j = x * m_j; c_j += sum_k t_j (pair tree)
            jsl = slice(jh * JH, (jh + 1) * JH)
            t_flat = t_all.rearrange("b j i k -> b (j i k)")
            for jj in range(JH):
                j = jh * JH + jj
                pm = ps.tile([B, M_SL, 512], f32, tag="pm", name="pm")
                for sl in range(M_SL):
                    nc.tensor.matmul(
                        pm[:, sl, 0:M_FREE],
                        lhsT=v_T[:, j, :],
                        rhs=wd_f[:, j, M_FREE * sl:M_FREE * (sl + 1)],
                        start=True, stop=True,
                    )
                mode = T_MODE[j]
                t_out = t_flat[:, j * ni_l * DI:(j + 1) * ni_l * DI]
                if mode in "BC":
                    m_bf = sc.tile([B, M_SL, M_FREE], bf16, tag="m_bf",
                                   name="m_bf")
                    nc.scalar.copy(out=m_bf, in_=pm[:, :, 0:M_FREE])
                    if mode == "B":
                        _stt(nc.vector, out=t_out,
                             in0=m_bf.rearrange("b s e -> b (s e)"),
                             in1=x_ik, op1=AL.mult)
                    else:
                        nc.gpsimd.tensor_tensor(
                            out=t_out, in0=m_bf.rearrange("b s e -> b (s e)"),
                            in1=x_ik, op=AL.mult)
                else:
                    _stt(nc.vector,
                         out=t_out.rearrange("b (s e) -> b s e", s=M_SL),
                         in0=pm[:, :, 0:M_FREE],
                         in1=x_ik.rearrange("b (s e) -> b s e", s=M_SL),
                         op1=AL.mult)
            # k pair-tree: 8 -> 4 -> 2 -> (+c)
            th = t_all[:, jsl, :, :]
            _stt(nc.vector, out=th[:, :, :, 0:4], in0=th[:, :, :, 0:4],
                 in1=th[:, :, :, 4:8], op1=AL.add)
            _stt(nc.vector, out=th[:, :, :, 0:2], in0=th[:, :, :, 0:2],
                 in1=th[:, :, :, 2:4], op1=AL.add)
            cv = c_t[:, jsl, :]
            if first:
                _stt(nc.vector, out=cv, in0=th[:, :, :, 0],
                     in1=th[:, :, :, 1], op1=AL.add)
            else:
                _stt(nc.vector, out=cv, in0=cv, in1=th[:, :, :, 0], op1=AL.add)
                _stt(nc.vector, out=cv, in0=cv, in1=th[:, :, :, 1], op1=AL.add)
            # e = exp(c) for this half (feeds next round's softmax)
            nc.scalar.activation(out=e_bf[:, jsl, :], in_=cv, func=AF.Exp)

        def softmax_pre():
            # sigma = sum_j e (pair tree, all inputs stride-1), rin = 1/sigma
            e5 = e_bf
            _stt(nc.vector, out=sg1, in0=e5[:, 0:5, :], in1=e5[:, 5:10, :],
                 op1=AL.add)
            sg2 = sc.tile([B, 2, ni_l], f32, tag="sg2", name="sg2")
            _stt(nc.vector, out=sg2, in0=sg1[:, 0:2, :], in1=sg1[:, 2:4, :],
                 op1=AL.add)
            _stt(nc.vector, out=rin, in0=sg2[:, 0, :], in1=sg2[:, 1, :],
                 op1=AL.add)
            _stt(nc.vector, out=rin, in0=rin, in1=sg1[:, 4, :], op1=AL.add)
            nc.vector.reciprocal(out=rin, in_=rin)
            nc.vector.tensor_copy(out=rin_bf, in_=rin)
            # transposed 1/sigma: chunk0 plain, chunk1 k-replicated
            ptr = ps.tile([128, B], bf16, tag="tp", name="ptr", padded_shape=[128, 1024])
            nc.tensor.transpose(ptr, rin_bf[:, 0:N0], ident)
            nc.scalar.copy(out=rT0, in_=ptr)
            r1rep = sc.tile([B, N1 * DI], bf16, tag="r1rep", name="r1rep")
            nc.vector.tensor_copy(
                out=r1rep.rearrange("b (i k) -> b i k", i=N1),
                in_=rin_bf[:, N0:ni_l].unsqueeze(2).broadcast_to([B, N1, DI]))
            ptr = ps.tile([128, B], bf16, tag="tp", name="ptr", padded_shape=[128, 1024])
            nc.tensor.transpose(ptr, r1rep, ident)
            nc.vector.tensor_copy(out=rT1, in_=ptr)
            _stt(nc.vector, out=xs0, in0=x_P0,
                 in1=rT0.unsqueeze(1).broadcast_to([N0, DI, B]), op1=AL.mult)
            _stt(nc.vector, out=xs1, in0=x_P1, in1=rT1, op1=AL.mult)

        def s_half(jh):
            # returns psum tile with s partials for j in this half
            ps_h = ps.tile([B, JH, DO], f32, tag="pm", name="ps_h")
            for jj in range(JH):
                j = jh * JH + jj
                # eT chunk0: [128, B]; chunk1 k-replicated: [(i k), B]
                pe0 = ps.tile([128, B], bf16, tag="tp", name="pe0", padded_shape=[128, 1024])
                nc.tensor.transpose(pe0, e_bf[:, j, 0:N0], ident)
                e1r = sc.tile([B, N1 * DI], bf16, tag="e1r", name="e1r")
                ecp = nc.vector if j % 2 == 0 else nc.scalar
                if ecp is nc.vector:
                    ecp.tensor_copy(
                        out=e1r.rearrange("b (i k) -> b i k", i=N1),
                        in_=e_bf[:, j, N0:ni_l].unsqueeze(2).broadcast_to(
                            [B, N1, DI]))
                else:
                    ecp.copy(
                        out=e1r.rearrange("b (i k) -> b i k", i=N1),
                        in_=e_bf[:, j, N0:ni_l].unsqueeze(2).broadcast_to(
                            [B, N1, DI]))
                pe1 = ps.tile([128, B], bf16, tag="tp", name="pe1", padded_shape=[128, 1024])
                nc.tensor.transpose(pe1, e1r, ident)
                y0 = sc.tile([N0, DI, B], bf16, tag="y0", name="y0")
                y1 = sc.tile([N1 * DI, B], bf16, tag="y1", name="y1")
                if j in Y_POOL_J:
                    e0s = sc.tile([N0, B], bf16, tag="e0s", name="e0s")
                    nc.scalar.copy(out=e0s, in_=pe0)
                    nc.gpsimd.tensor_tensor(
                        out=y0, in0=xs0,
                        in1=e0s.unsqueeze(1).broadcast_to([N0, DI, B]),
                        op=AL.mult)
                else:
                    _stt(nc.vector, out=y0, in0=xs0,
                         in1=pe0.unsqueeze(1).broadcast_to([N0, DI, B]),
                         op1=AL.mult)
                _stt(nc.vector, out=y1, in0=xs1, in1=pe1, op1=AL.mult)
                nc.tensor.matmul(ps_h[:, jj, :], lhsT=y1, rhs=w1[:, j, :],
                                 start=True, stop=False)
                for k in range(DI):
                    nc.tensor.matmul(
                        ps_h[:, jj, :],
                        lhsT=y0[:, k, :],
                        rhs=w0[:, j, :, k],
                        start=False, stop=(k == DI - 1),
                    )
            return ps_h

        # ---------------- r0: s0 = (1/NJ) * sum_ik x W  (scale in squash) ---
        ps0 = ps.tile([B, NJ, DO], f32, tag="pm", name="ps0")
        nc.tensor.matmul(ps0.rearrange("b j d -> b (j d)"), lhsT=x_P1,
                         rhs=w1.rearrange("p j d -> p (j d)"),
                         start=True, stop=False)
        for k in range(DI):
            nc.tensor.matmul(
                ps0.rearrange("b j d -> b (j d)"),
                lhsT=x_P0[:, k, :],
                rhs=w0[:, :, :, k].rearrange("i j d -> i (j d)"),
                start=False, stop=(k == DI - 1),
            )
        nc.scalar.copy(out=s_part_f, in_=ps0.rearrange("b j d -> b (j d)"))
        nc.sync.dma_start(out=arf_in, in_=s_part_f)
        if single:
            nc.sync.dma_start(out=arf_out, in_=arf_in)
        else:
            nc.gpsimd.collective_compute(
                "AllReduce", AL.add,
                ins=[arf_in.opt()], outs=[arf_out.opt()],
                replica_groups=[list(range(n_cores))],
            )
        nc.sync.dma_start(out=s_full.rearrange("b j d -> b (j d)"), in_=arf_out)
        for jh in range(2):
            squash_h(jh, r0=True)
            vT_h(jh)
            c_update_h(jh, first=True)

        # ---------------- routing rounds ----------------
        for r in range(1, ROUTINGS):
            last = (r == ROUTINGS - 1)
            softmax_pre()
            for jh in range(2):
                ps_h = s_half(jh)
                allreduce_h(jh, ps_h)
            for jh in range(2):
                squash_h(jh, last=last)
                if not last:
                    vT_h(jh)
                    c_update_h(jh, first=False)

        nc.sync.dma_start(out=out_d.ap(), in_=v_out)


_NC_CACHE = {}


def kernel(inputs: np.ndarray, W: np.ndarray) -> np.ndarray:
    import ml_dtypes
    bf = ml_dtypes.bfloat16
    n_cores = 8
    ni_l = NI // n_cores
    if "nc" not in _NC_CACHE:
        _NC_CACHE["nc"] = build_kernel(n_cores=n_cores, debug=False)
    nc = _NC_CACHE["nc"]
    in_maps = []
    for r in range(n_cores):
        sl = slice(ni_l * r, ni_l * (r + 1))
        Wl = np.ascontiguousarray(W[:, sl], dtype=np.float32)  # [NJ,ni_l,DO,DI]
        w0 = np.ascontiguousarray(
            Wl[:, 0:N0].transpose(1, 0, 2, 3)).astype(bf)      # [N0,NJ,DO,DI]
        w1 = np.ascontiguousarray(
            Wl[:, N0:ni_l].transpose(1, 3, 0, 2).reshape(
                N1 * DI, NJ, DO)).astype(bf)                   # [(i k),NJ,DO]
        wdl = np.ascontiguousarray(Wl.transpose(2, 0, 1, 3))   # [DO,NJ,ni_l,DI]
        in_maps.append({
            "x": np.ascontiguousarray(inputs[:, sl, :]).astype(bf),
            "w0": w0,
            "w1": w1,
            "wd": wdl,
        })
    res = run_bass_kernel_spmd(nc, in_maps, core_ids=list(range(n_cores)))
    return res.results[0]["out"]


# revision 18
# speedup vs baseline: 1.3110x; 1.1062x over previous
"""Trainium2 Bass kernel for nn_CapsuleLayer (dynamic routing capsule layer).

Sharding: the 1152 input capsules (i) are split across 8 cores (144 each);
the full batch B=128 lives on SBUF partitions. Routing state (c, p) stays
local to each core's i-shard; per-iteration s partial sums are combined with
small AllReduces ([128,80] f32, one per j-half so the collective latency
hides under the other half's compute). u_hat is never materialized:
  s[b,j,d]       = sum_{i,k} (exp(c)/sigma)[b,j,i] x[b,i,k] W[j,i,d,k]  (PE)
  c_delta[b,j,i] = sum_k x[b,i,k] m[b,j,i,k],   m = sum_d v[b,j,d] W[j,i,d,k]
Host pre-lays W out in three device layouts (i-major bf16 for s, (i*k)-packed
bf16 for the 16-capsule remainder chunk, d-major f32 for m) so the device
does no staging casts. All activation funcs (exp, ln for rsqrt) come from
one table set, so LoadActFuncSet is emitted once.
"""

import sys

if "/opt/trn_rl_repo" not in sys.path:
    sys.path.insert(0, "/opt/trn_rl_repo")

import contextlib

import numpy as np

import concourse.bass as bass  # noqa: F401
import concourse.tile as tile
from concourse import bacc, mybir
from concourse.bass_utils import run_bass_kernel_spmd
from concourse.masks import make_identity


def _steer_act_tables():
    # Compile-time table-choice steering: strip the funcs we use (exp/ln/
    # copy/identity) from every set that precedes natural_log_exp_and_others
    # so the selector lands all activations on that one set -> a single
    # LoadActFuncSet for the whole kernel. Indices of the sets are preserved,
    # so the emitted act_func_set_id still matches act_info.json.
    from concourse import bacc as _bacc_mod
    import concourse.hw_specs as _hws
    if getattr(_steer_act_tables, "_done", False):
        return
    _orig = _hws.get_activation_tables

    def _patched(arch):
        tabs = _orig(arch)
        target = "natural_log_exp_and_others"
        if target not in tabs:
            return tabs
        keep = tabs[target]
        out = {}
        for name, funcs in tabs.items():
            out[name] = funcs if name == target else (funcs - keep)
        return out

    _bacc_mod.get_activation_tables = _patched
    _steer_act_tables._done = True

f32 = mybir.dt.float32
f32r = mybir.dt.float32r
bf16 = mybir.dt.bfloat16
AL = mybir.AluOpType
AF = mybir.ActivationFunctionType

B = 128          # batch (on partitions)
NJ = 10          # output capsules
DO = 16          # output capsule dim
DI = 8           # input capsule dim
NI = 1152        # input capsules (global)
ROUTINGS = 3
EPS = 1e-7

N0 = 128         # chunk0: i = 0..127, i on partitions
N1 = 16          # chunk1: i = 128..143, (i,k) packed on partitions (16*8=128)
M_SL = 3         # m-matmul free-dim slices (3 x 384 = 1152)
M_FREE = (N0 + N1) * DI // M_SL

# engine assignment knobs (tuned against TimelineSim)
# t-multiply mode per j: A = DVE direct from PSUM, B = Act evac + DVE mult,
# D = Pool direct from PSUM
T_MODE = "ABBCB" * 2
Y_POOL_J = (2, 5, 8)         # j's whose chunk0 y-multiply runs on Pool (from an Act-evac'd SBUF copy; GPSIMD cannot read PSUM)


def _stt(eng, out, in0, in1, op1, scalar=None, op0=None):
    if scalar is not None and scalar != 1.0 or op0 is not None and op0 != AL.mult:
        eng.scalar_tensor_tensor(out=out, in0=in0, scalar=scalar or 1.0,
                                 in1=in1, op0=op0 or AL.mult, op1=op1)
    else:
        eng.tensor_tensor(out=out, in0=in0, in1=in1, op=op1)


def build_kernel(n_cores=8, debug=False, repeat=1, single=False, ablate=()):
    ni_l = NI // n_cores
    assert ni_l == N0 + N1

    _steer_act_tables()
    nc = bacc.Bacc("TRN2", target_bir_lowering=False, debug=False,
                   num_devices=1 if single else n_cores)
    # host-prepped layouts (see kernel() below)
    x_d = nc.dram_tensor("x", [B, ni_l, DI], bf16, kind="ExternalInput")
    w0_d = nc.dram_tensor("w0", [N0, NJ, DO, DI], bf16, kind="ExternalInput")
    w1_d = nc.dram_tensor("w1", [N1 * DI, NJ, DO], bf16, kind="ExternalInput")
    wd_d = nc.dram_tensor("wd", [DO, NJ, ni_l, DI], f32, kind="ExternalInput")
    out_d = nc.dram_tensor("out", [B, NJ, DO], f32, kind="ExternalOutput")

    with tile.TileContext(nc) as tc:
        for _rep in range(repeat):
            _body(nc, tc, x_d, w0_d, w1_d, wd_d, out_d, ni_l, n_cores, single)
    nc.compile()
    return nc


def _body(nc, tc, x_d, w0_d, w1_d, wd_d, out_d, ni_l, n_cores, single=False):
    ctx = contextlib.ExitStack()
    JH = NJ // 2
    with ctx:
        sb = ctx.enter_context(tc.tile_pool(name="sb", bufs=1))
        sc = ctx.enter_context(tc.tile_pool(name="scratch", bufs=2))
        ps = ctx.enter_context(tc.tile_pool(name="ps", bufs=2, space="PSUM"))
        dram = ctx.enter_context(tc.tile_pool(name="dram", bufs=2, space="DRAM"))

        # ---------------- loads (no staging: host sent final layouts) -------
        x_bf = sb.tile([B, ni_l, DI], bf16)
        nc.sync.dma_start(out=x_bf.rearrange("b i k -> b (i k)"),
                          in_=x_d.ap().rearrange("b i k -> b (i k)"))
        w0 = sb.tile([N0, NJ, DO, DI], bf16)
        nc.sync.dma_start(out=w0.rearrange("i j d k -> i (j d k)"),
                          in_=w0_d.ap().rearrange("i j d k -> i (j d k)"))
        w1 = sb.tile([N1 * DI, NJ, DO], bf16)
        nc.sync.dma_start(out=w1.rearrange("p j d -> p (j d)"),
                          in_=w1_d.ap().rearrange("p j d -> p (j d)"))
        wd_r = sb.tile([DO, NJ, ni_l, DI], f32r, name="wd_r")
        nc.sync.dma_start(
            out=wd_r.rearrange("d j i k -> d (j i k)"),
            in_=wd_d.ap().rearrange("d j i k -> d (j i k)").bitcast(f32r))

        ident = sb.tile([128, 128], bf16)
        make_identity(nc, ident)
        ident_f = sb.tile([128, 128], f32)
        make_identity(nc, ident_f)

        # x_P0: [i0, k, b]; x_P1: [(i1 k), b]  (PE transposes via PSUM)
        x_P0 = sb.tile([N0, DI, B], bf16)
        x_P1 = sb.tile([N1 * DI, B], bf16)
        for k in range(DI):
            pt = ps.tile([128, B], bf16, tag="tp", name="pt", padded_shape=[128, 1024])
            nc.tensor.transpose(pt, x_bf[:, 0:N0, k], ident)
            eng = nc.vector if k % 2 == 0 else nc.scalar
            if eng is nc.scalar:
                eng.copy(out=x_P0[:, k, :], in_=pt)
            else:
                eng.tensor_copy(out=x_P0[:, k, :], in_=pt)
        pt = ps.tile([128, B], bf16, tag="tp", name="pt", padded_shape=[128, 1024])
        nc.tensor.transpose(
            pt, x_bf[:, N0:ni_l, :].rearrange("b i k -> b (i k)"), ident)
        nc.vector.tensor_copy(out=x_P1, in_=pt)

        # persistent state
        c_t = sb.tile([B, NJ, ni_l], f32)        # routing logits
        s_full = sb.tile([B, NJ, DO], f32)       # all-reduced s (j,d)
        v_jd = sb.tile([B, NJ, DO], f32)         # squashed v (j,d)
        v_T = sb.tile([DO, NJ, B], f32r)         # v transposed [d, j, b]
        e_bf = sb.tile([B, NJ, ni_l], bf16)      # exp(c)
        sg1 = sb.tile([B, 5, ni_l], f32)         # sigma tree scratch
        rin = sb.tile([B, ni_l], f32)            # 1/sigma
        rin_bf = sb.tile([B, ni_l], bf16)
        rT0 = sb.tile([N0, B], bf16)
        rT1 = sb.tile([N1 * DI, B], bf16)
        xs0 = sb.tile([N0, DI, B], bf16)         # x * (1/sigma), chunk0
        xs1 = sb.tile([N1 * DI, B], bf16)
        t_all = sb.tile([B, NJ, ni_l, DI], bf16)  # m * x scratch
        sq = sb.tile([B, NJ], f32)
        fac = sb.tile([B, NJ], f32)
        v_out = sb.tile([B, NJ, DO], f32)
        eps_t = sb.tile([B, 1], f32)
        nc.vector.memset(eps_t, EPS)

        ar_in = [dram.tile([B, JH * DO], f32, tag=f"ai{h}", name=f"ai{h}") for h in range(2)]
        ar_out = [dram.tile([B, JH * DO], f32, tag=f"ao{h}", name=f"ao{h}") for h in range(2)]
        arf_in = dram.tile([B, NJ * DO], f32, tag="afi", name="afi")
        arf_out = dram.tile([B, NJ * DO], f32, tag="afo", name="afo")
        s_part_f = sb.tile([B, NJ * DO], f32)

        s_part = [sb.tile([B, JH * DO], f32, tag=f"sp{h}", name=f"sp{h}")
                  for h in range(2)]

        def allreduce_h(jh, ps_h):
            # PSUM -> SBUF -> (collective, SBUF to SBUF) -> half of s_full
            nc.scalar.copy(out=s_part[jh],
                           in_=ps_h.rearrange("b j d -> b (j d)"))
            jsl = slice(jh * JH, (jh + 1) * JH)
            dst = s_full[:, jsl, :].rearrange("b j d -> b (j d)")
            if single:
                nc.sync.dma_start(out=dst, in_=s_part[jh])
            else:
                nc.gpsimd.collective_compute(
                    "AllReduce", AL.add,
                    ins=[s_part[jh].opt()], outs=[dst.opt()],
                    replica_groups=[list(range(n_cores))],
                )

        def squash_h(jh, r0=False, last=False):
            # v = s*inv_scale * g/(1+g) / sqrt(g+eps), g = ||s*inv_scale||^2
            # rsqrt via exp(-0.5*ln(g+eps)) -- same act table as exp.
            jsl = slice(jh * JH, (jh + 1) * JH)
            sc2 = (1.0 / (NJ * NJ)) if r0 else 1.0
            sc1 = (1.0 / NJ) if r0 else 1.0
            s_h = s_full[:, jsl, :]
            t = sc.tile([B, JH, DO], f32, tag="sqt", name="t")
            _stt(nc.vector, out=t, in0=s_h, in1=s_h, op1=AL.mult, scalar=sc2)
            g = sq[:, jsl]
            nc.vector.tensor_reduce(out=g, in_=t, axis=mybir.AxisListType.X,
                                    op=AL.add)
            lg = sc.tile([B, JH], f32, tag="lg", name="lg")
            nc.scalar.activation(out=lg, in_=g, func=AF.Ln, bias=eps_t)
            rs = sc.tile([B, JH], f32, tag="rs", name="rs")
            nc.scalar.activation(out=rs, in_=lg, func=AF.Exp, scale=-0.5)
            den = sc.tile([B, JH], f32, tag="den", name="den")
            nc.vector.tensor_scalar(out=den, in0=g, scalar1=1.0, scalar2=None,
                                    op0=AL.add)
            nc.vector.reciprocal(out=den, in_=den)
            gr = sc.tile([B, JH], f32, tag="gr", name="gr")
            nc.vector.tensor_tensor(out=gr, in0=g, in1=rs, op=AL.mult)
            f_h = fac[:, jsl]
            nc.vector.tensor_tensor(out=f_h, in0=gr, in1=den, op=AL.mult)
            out_t = v_out if last else v_jd
            _stt(nc.vector, out=out_t[:, jsl, :], in0=s_h,
                 in1=f_h.unsqueeze(2).broadcast_to([B, JH, DO]),
                 op1=AL.mult, scalar=sc1)

        def vT_j(j, jj):
            ptv = ps.tile([128, B], f32, tag="tp", name="ptv",
                          padded_shape=[128, 512])
            nc.tensor.transpose(ptv[:DO, :], v_jd[:, j, :], ident_f)
            if jj % 2 == 0:
                nc.vector.tensor_copy(out=v_T[:, j, :], in_=ptv[:DO, :])
            else:
                nc.scalar.copy(out=v_T[:, j, :], in_=ptv[:DO, :])

        wd_f = wd_r.rearrange("d j i k -> d j (i k)")
        x_ik = x_bf.rearrange("b i k -> b (i k)")

        def c_update_h(jh, first):
            # per j: v transpose -> m = v W (PE) -> t = x*m; then k pair-tree
            jsl = slice(jh * JH, (jh + 1) * JH)
            t_flat = t_all.rearrange("b j i k -> b (j i k)")
            for jj in range(JH):
                j = jh * JH + jj
                vT_j(j, jj)
                pm = ps.tile([B, M_SL, 512], f32, tag="pm", name="pm")
                for sl in range(M_SL):
                    nc.tensor.matmul(
                        pm[:, sl, 0:M_FREE],
                        lhsT=v_T[:, j, :],
                        rhs=wd_f[:, j, M_FREE * sl:M_FREE * (sl + 1)],
                        start=True, stop=True,
                    )
                mode = T_MODE[j]
                t_out = t_flat[:, j * ni_l * DI:(j + 1) * ni_l * DI]
                if mode in "BC":
                    m_bf = sc.tile([B, M_SL, M_FREE], bf16, tag="m_bf",
                                   name="m_bf")
                    nc.scalar.copy(out=m_bf, in_=pm[:, :, 0:M_FREE])
                    if mode == "B":
                        _stt(nc.vector, out=t_out,
                             in0=m_bf.rearrange("b s e -> b (s e)"),
                             in1=x_ik, op1=AL.mult)
                    else:
                        nc.gpsimd.tensor_tensor(
                            out=t_out, in0=m_bf.rearrange("b s e -> b (s e)"),
                            in1=x_ik, op=AL.mult)
                else:
                    _stt(nc.vector,
                         out=t_out.rearrange("b (s e) -> b s e", s=M_SL),
                         in0=pm[:, :, 0:M_FREE],
                         in1=x_ik.rearrange("b (s e) -> b s e", s=M_SL),
                         op1=AL.mult)
            # k pair-tree: 8 -> 4 -> 2 -> (+c)
            th = t_all[:, jsl, :, :]
            _stt(nc.vector, out=th[:, :, :, 0:4], in0=th[:, :, :, 0:4],
                 in1=th[:, :, :, 4:8], op1=AL.add)
            _stt(nc.vector, out=th[:, :, :, 0:2], in0=th[:, :, :, 0:2],
                 in1=th[:, :, :, 2:4], op1=AL.add)
            cv = c_t[:, jsl, :]
            if first:
                _stt(nc.vector, out=cv, in0=th[:, :, :, 0],
                     in1=th[:, :, :, 1], op1=AL.add)
            else:
                _stt(nc.vector, out=cv, in0=cv, in1=th[:, :, :, 0], op1=AL.add)
                _stt(nc.vector, out=cv, in0=cv, in1=th[:, :, :, 1], op1=AL.add)
            # e = exp(c) for this half (feeds next round's softmax)
            nc.scalar.activation(out=e_bf[:, jsl, :], in_=cv, func=AF.Exp)

        def softmax_pre():
            # sigma = sum_j e (pair tree, all inputs stride-1), rin = 1/sigma
            e5 = e_bf
            _stt(nc.vector, out=sg1, in0=e5[:, 0:5, :], in1=e5[:, 5:10, :],
                 op1=AL.add)
            sg2 = sc.tile([B, 2, ni_l], f32, tag="sg2", name="sg2")
            _stt(nc.vector, out=sg2, in0=sg1[:, 0:2, :], in1=sg1[:, 2:4, :],
                 op1=AL.add)
            _stt(nc.vector, out=rin, in0=sg2[:, 0, :], in1=sg2[:, 1, :],
                 op1=AL.add)
            _stt(nc.vector, out=rin, in0=rin, in1=sg1[:, 4, :], op1=AL.add)
            nc.vector.reciprocal(out=rin, in_=rin)
            nc.vector.tensor_copy(out=rin_bf, in_=rin)
            # transposed 1/sigma: chunk0 plain, chunk1 k-replicated
            ptr = ps.tile([128, B], bf16, tag="tp", name="ptr", padded_shape=[128, 1024])
            nc.tensor.transpose(ptr, rin_bf[:, 0:N0], ident)
            nc.scalar.copy(out=rT0, in_=ptr)
            r1rep = sc.tile([B, N1 * DI], bf16, tag="r1rep", name="r1rep")
            nc.vector.tensor_copy(
                out=r1rep.rearrange("b (i k) -> b i k", i=N1),
                in_=rin_bf[:, N0:ni_l].unsqueeze(2).broadcast_to([B, N1, DI]))
            ptr = ps.tile([128, B], bf16, tag="tp", name="ptr", padded_shape=[128, 1024])
            nc.tensor.transpose(ptr, r1rep, ident)
            nc.vector.tensor_copy(out=rT1, in_=ptr)
            _stt(nc.vector, out=xs0, in0=x_P0,
                 in1=rT0.unsqueeze(1).broadcast_to([N0, DI, B]), op1=AL.mult)
            _stt(nc.vector, out=xs1, in0=x_P1, in1=rT1, op1=AL.mult)
            nc.vector.tensor_copy(
                out=e1r_all.rearrange("b j (i k) -> b j i k", i=N1),
                in_=e_bf[:, :, N0:ni_l].unsqueeze(3).broadcast_to(
                    [B, NJ, N1, DI]))

        e1r_all = sb.tile([B, NJ, N1 * DI], bf16)

        def s_half(jh):
            # returns psum tile with s partials for j in this half
            ps_h = ps.tile([B, JH, DO], f32, tag="pm", name="ps_h")
            for jj in range(JH):
                j = jh * JH + jj
                # eT chunk0: [128, B]; chunk1 k-replicated: [(i k), B]
                pe0 = ps.tile([128, B], bf16, tag="tp", name="pe0", padded_shape=[128, 1024])
                nc.tensor.transpose(pe0, e_bf[:, j, 0:N0], ident)
                pe1 = ps.tile([128, B], bf16, tag="tp", name="pe1", padded_shape=[128, 1024])
                nc.tensor.transpose(pe1, e1r_all[:, j, :], ident)
                y0 = sc.tile([N0, DI, B], bf16, tag="y0", name="y0")
                y1 = sc.tile([N1 * DI, B], bf16, tag="y1", name="y1")
                if j in Y_POOL_J:
                    e0s = sc.tile([N0, B], bf16, tag="e0s", name="e0s")
                    nc.scalar.copy(out=e0s, in_=pe0)
                    nc.gpsimd.tensor_tensor(
                        out=y0, in0=xs0,
                        in1=e0s.unsqueeze(1).broadcast_to([N0, DI, B]),
                        op=AL.mult)
                else:
                    _stt(nc.vector, out=y0, in0=xs0,
                         in1=pe0.unsqueeze(1).broadcast_to([N0, DI, B]),
                         op1=AL.mult)
                _stt(nc.vector, out=y1, in0=xs1, in1=pe1, op1=AL.mult)
                nc.tensor.matmul(ps_h[:, jj, :], lhsT=y1, rhs=w1[:, j, :],
                                 start=True, stop=False)
                for k in range(DI):
                    nc.tensor.matmul(
                        ps_h[:, jj, :],
                        lhsT=y0[:, k, :],
                        rhs=w0[:, j, :, k],
                        start=False, stop=(k == DI - 1),
                    )
            return ps_h

        # ---------------- r0: s0 = (1/NJ) * sum_ik x W  (scale in squash) ---
        ps0 = ps.tile([B, NJ, DO], f32, tag="pm", name="ps0")
        nc.tensor.matmul(ps0.rearrange("b j d -> b (j d)"), lhsT=x_P1,
                         rhs=w1.rearrange("p j d -> p (j d)"),
                         start=True, stop=False)
        for k in range(DI):
            nc.tensor.matmul(
                ps0.rearrange("b j d -> b (j d)"),
                lhsT=x_P0[:, k, :],
                rhs=w0[:, :, :, k].rearrange("i j d -> i (j d)"),
                start=False, stop=(k == DI - 1),
            )
        nc.scalar.copy(out=s_part_f, in_=ps0.rearrange("b j d -> b (j d)"))
        dstf = s_full.rearrange("b j d -> b (j d)")
        if single:
            nc.sync.dma_start(out=dstf, in_=s_part_f)
        else:
            nc.gpsimd.collective_compute(
                "AllReduce", AL.add,
                ins=[s_part_f.opt()], outs=[dstf.opt()],
                replica_groups=[list(range(n_cores))],
            )
        for jh in range(2):
            squash_h(jh, r0=True)
            c_update_h(jh, first=True)

        # ---------------- routing rounds ----------------
        for r in range(1, ROUTINGS):
            last = (r == ROUTINGS - 1)
            softmax_pre()
            for jh in range(2):
                ps_h = s_half(jh)
                allreduce_h(jh, ps_h)
            for jh in range(2):
                squash_h(jh, last=last)
                if not last:
                    c_update_h(jh, first=False)

        nc.sync.dma_start(out=out_d.ap(), in_=v_out)


_NC_CACHE = {}


def kernel(inputs: np.ndarray, W: np.ndarray) -> np.ndarray:
    import ml_dtypes
    bf = ml_dtypes.bfloat16
    n_cores = 8
    ni_l = NI // n_cores
    if "nc" not in _NC_CACHE:
        _NC_CACHE["nc"] = build_kernel(n_cores=n_cores, debug=False)
    nc = _NC_CACHE["nc"]
    in_maps = []
    for r in range(n_cores):
        sl = slice(ni_l * r, ni_l * (r + 1))
        Wl = np.ascontiguousarray(W[:, sl], dtype=np.float32)  # [NJ,ni_l,DO,DI]
        w0 = np.ascontiguousarray(
            Wl[:, 0:N0].transpose(1, 0, 2, 3)).astype(bf)      # [N0,NJ,DO,DI]
        w1 = np.ascontiguousarray(
            Wl[:, N0:ni_l].transpose(1, 3, 0, 2).reshape(
                N1 * DI, NJ, DO)).astype(bf)                   # [(i k),NJ,DO]
        wdl = np.ascontiguousarray(Wl.transpose(2, 0, 1, 3))   # [DO,NJ,ni_l,DI]
        in_maps.append({
            "x": np.ascontiguousarray(inputs[:, sl, :]).astype(bf),
            "w0": w0,
            "w1": w1,
            "wd": wdl,
        })
    res = run_bass_kernel_spmd(nc, in_maps, core_ids=list(range(n_cores)))
    return res.results[0]["out"]
